# revision 23
# baseline (speedup 1.0000x reference)
"""2-layer GCN (DGL GraphConv norm='both') on 8 Trainium2 NeuronCores.

v5 strategy (dst-sharded layer 1, src-sharded scatter-add layer 2):
  - Host: degree norms; per-shard nodes PERMUTED by out-degree key
    max(d_lo, d_hi); L1 edges packed into 128-edge tiles in WINDOW-MAJOR
    chunk order (per ~6-window chunk: all lo-half tiles, then hi-half),
    so each dst window's psum accumulates lo+hi back-to-back and closes
    early -- no cross-pass partial staging (x1acc) needed, and layer-2
    scatters interleave across the whole L1 phase.
  - L2 edges colored into layers: per layer at most one edge per src
    rank (block-prefix of ranks) and all-distinct dst rows, because
    dma_scatter_add races on same-row descriptors within one call.
    One scatter call per layer piece (wide layers split for earlier
    eligibility); idx streams pad holes with a garbage row.
  - Device per core c:
      h_sh = XnT_sh.T @ W1; AllGather h -> [50176, 128] bf16 table
      window-major aggregation: dma_gather rows + onehot matmuls
      window epilogue: x1n = relu((agg1*ndst + b1))*nsrc (bf16),
        transpose, x2win = x1n @ W2 -> x2stage [128, 49*8] f32;
        then emit eligible scatter calls (32B f32 payloads into a
        [50176, 64] f32 table, 256B row stride)
      ReduceScatter(add) -> rs_out [6272, 64]; out = rs * ndst
  - Host: un-permute rows, add b2, concat cores.
"""

import os
import numpy as np
import ml_dtypes

import concourse.bass as bass
import concourse.bacc as bacc
import concourse.mybir as mybir
import concourse.tile as tile
from concourse import bass_utils

BF16 = ml_dtypes.bfloat16

N = 50000
E = 1600000
FIN = 1433
FP = 1536            # FIN padded to 12*128
H = 128
C = 7
NCORES = 8
NSH = N // NCORES    # 6250
W = 64               # dst window width
NW = (NSH + W - 1) // W          # 98 windows per core
NSHP = NW * W                    # 6272 padded shard rows
TROWS = NCORES * NSHP            # 50176 table rows
LO = 32768                       # lo/hi h-table split (gather idx int16)
SPLIT = TROWS // 2               # 25088 scatter-table half split
BT = 72              # max tiles per dma_gather call
SUPER = 5            # windows per L1 chunk
PAD_DST = 200.0      # local-dst sentinel for pad edges (> W-1)
GARBAGE = 6271       # scatter pad row (pad rank of core 0 / core 4)
STEP = 64            # scatter table row stride (f32) = 256B

KB = NW * W // 128   # 49 node blocks of 128 in dense stage
KCH = FP // 128      # 12 contraction chunks
GP_BUFS = 2          # gather-tile buffering
OH_BUFS = 2          # onehot-tile buffering
PSW_BUFS = SUPER + 1  # window psum ring (shared with dense stage)
NSB = 8              # node blocks per xnt load in dense stage
NSWQ = 1             # SWDGE queues


def _ceil_div(a, b):
    return (a + b - 1) // b


def _wrap_idx(idx_flat):
    """[n] -> [128, n//16] int16: desc i -> [i%16 (+16k copies), i//16]."""
    a = np.asarray(idx_flat, np.int16).reshape(-1, 16).T
    return np.ascontiguousarray(np.tile(a, (8, 1)))


def _color_layers(rks, rows, seed=1):
    """Assign each edge (src rank, dst row) a layer: per layer, at most one
    edge per rank and all-distinct rows.  Ranks are degree-sorted, so rank
    r's edges go to layers 0..deg(r)-1 bijectively; iterative random swaps
    repair row conflicts; stubborn leftovers go to fresh layers."""
    rng = np.random.default_rng(seed)
    n = len(rks)
    o = np.argsort(rks, kind="stable")
    rks, rows = rks[o], rows[o]
    starts = np.searchsorted(rks, np.arange(NSH + 1))
    layer = np.arange(n) - starts[rks]
    for _ in range(300):
        key = layer.astype(np.int64) * (SPLIT + 1) + rows
        order = np.argsort(key, kind="stable")
        ks = key[order]
        dup = np.zeros(n, bool)
        dup[order[1:]] = ks[1:] == ks[:-1]
        if not dup.any():
            break
        idx = np.nonzero(dup)[0]
        rng.shuffle(idx)
        for i in idx:
            r = rks[i]
            j = rng.integers(starts[r], starts[r + 1])
            layer[i], layer[j] = layer[j], layer[i]
    else:
        # leftovers -> fresh layers; both rows AND ranks must stay unique
        key = layer.astype(np.int64) * (SPLIT + 1) + rows
        order = np.argsort(key, kind="stable")
        ks = key[order]
        dup = np.zeros(n, bool)
        dup[order[1:]] = ks[1:] == ks[:-1]
        nl = int(layer.max()) + 1
        used_rows = {}
        used_rks = {}
        for i in np.nonzero(dup)[0]:
            j = nl
            while (rows[i] in used_rows.setdefault(j, set())
                   or rks[i] in used_rks.setdefault(j, set())):
                j += 1
            used_rows[j].add(rows[i])
            used_rks[j].add(rks[i])
            layer[i] = j
    return rks, rows, layer


def _prep(features, src, dst, W1, b1, W2, b2):
    """Host-side sharding/packing. Returns (in_maps, program-shape params)."""
    src = np.asarray(src).astype(np.int64)
    dst = np.asarray(dst).astype(np.int64)
    features = np.asarray(features, np.float32)

    deg_src = np.bincount(src, minlength=N).astype(np.float32)
    deg_dst = np.bincount(dst, minlength=N).astype(np.float32)
    nsrc = 1.0 / np.sqrt(np.maximum(deg_src, 1.0))
    ndst = 1.0 / np.sqrt(np.maximum(deg_dst, 1.0))

    # ---- per-shard permutation by out-degree (lo/hi = dst core group) ----
    lo_edge = (dst // NSH) < (NCORES // 2)
    d_lo_g = np.bincount(src[lo_edge], minlength=N)
    d_hi_g = np.bincount(src[~lo_edge], minlength=N)
    perms = []
    rank_of_g = np.empty(N, np.int64)
    for c in range(NCORES):
        a, b = c * NSH, (c + 1) * NSH
        key = np.maximum(d_lo_g[a:b], d_hi_g[a:b])
        order = np.argsort(-key, kind="stable")     # rank -> orig local id
        perms.append(order)
        rank_of = np.empty(NSH, np.int64)
        rank_of[order] = np.arange(NSH)
        rank_of_g[a:b] = rank_of
    grank = (np.arange(N) // NSH) * NSHP + rank_of_g    # node -> table row

    g_src = grank[src]
    g_dst = grank[dst]

    # ---- L1 edge tiles (dst-sharded, window-major chunk order) ----
    dcore = dst // NSH
    dloc = rank_of_g[dst]
    win = dloc // W
    half = (g_src >= LO).astype(np.int64)

    cnt = np.zeros((NCORES, NW, 2), np.int64)
    per_core = []
    for c in range(NCORES):
        m = dcore == c
        gs, wn, hf, dl = g_src[m], win[m], half[m], dloc[m]
        order = np.lexsort((gs, hf, wn))
        gs, wn, hf, dl = gs[order], wn[order], hf[order], dl[order]
        key = wn * 2 + hf
        cnt[c] = np.bincount(key, minlength=NW * 2).reshape(NW, 2)
        per_core.append((gs, dl, key))

    tw = np.zeros((NW, 2), np.int64)
    for h in range(2):
        tw[:, h] = _ceil_div(np.max(cnt[:, :, h], axis=0), 128)

    # window-major chunk order: per SUPER-window chunk, lo tiles then hi
    tile_win = []
    tile_base = np.zeros((NW, 2), np.int64)
    gcalls = []           # (half, tile_start, ntiles)
    for c0 in range(0, NW, SUPER):
        ws = range(c0, min(c0 + SUPER, NW))
        for h in (0, 1):
            t0 = len(tile_win)
            for w in ws:
                tile_base[w, h] = len(tile_win)
                tile_win.extend([w] * int(tw[w, h]))
            n = len(tile_win) - t0
            while n > 0:
                k = min(BT, n)
                gcalls.append((h, t0, k))
                t0 += k
                n -= k
    T = len(tile_win)

    # ---- L2 scatter layers (src-sharded, collision-free per call) ----
    score = src // NSH
    s_half = (g_dst >= SPLIT).astype(np.int64)
    s_row = g_dst - s_half * SPLIT
    core_layers = []      # [c][h] -> (rks, rows, layer)
    nl = np.zeros((NCORES, 2), np.int64)
    for c in range(NCORES):
        m = score == c
        rk_c = rank_of_g[src[m]]
        hf_c = s_half[m]
        rw_c = s_row[m]
        res = []
        for h in (0, 1):
            mm = hf_c == h
            rks, rows, layer = _color_layers(rk_c[mm], rw_c[mm],
                                             seed=17 * c + h)
            res.append((rks, rows, layer))
            nl[c, h] = layer.max() + 1
        core_layers.append(res)

    # shared schedule: per (half, layer) block count = max over cores,
    # wide layers split so the first piece can fire mid-L1; entries
    # ordered by eligibility (blk1), alternating halves to break the
    # per-half WAW chain between scatter calls
    raw = []              # [(half, layer, B)]
    for h in (0, 1):
        nlh = int(nl[:, h].max())
        for j in range(nlh):
            B = 0
            for c in range(NCORES):
                rks, rows, layer = core_layers[c][h]
                mj = layer == j
                if mj.any():
                    B = max(B, int(rks[mj].max()) // 128 + 1)
            raw.append((h, j, B))
    entries = []          # [(half, layer, blk0, blk1)]
    for h, j, B in raw:
        if B > 40:
            entries.append((h, j, 0, 25))
            entries.append((h, j, 25, 37))
            entries.append((h, j, 37, B))
        elif B > 32:
            entries.append((h, j, 0, 25))
            entries.append((h, j, 25, B))
        else:
            entries.append((h, j, 0, B))
    entries.sort(key=lambda e: (e[3], e[0], e[1]))
    sched = []
    i = 0
    while i < len(entries):
        k = i
        while k < len(entries) and entries[k][3] == entries[i][3]:
            k += 1
        group = entries[i:k]
        los = [e for e in group if e[0] == 0]
        his = [e for e in group if e[0] == 1]
        while los or his:
            if los:
                sched.append(los.pop(0))
            if his:
                sched.append(his.pop(0))
        i = k
    ndesc = sum((e[3] - e[2]) * 128 for e in sched)

    # per-core scatter idx streams, in sched order
    idx2s = []
    for c in range(NCORES):
        stream = np.full(ndesc, GARBAGE, np.int16)
        off = 0
        for sh, sj, sb0_, sb1_ in sched:
            n = (sb1_ - sb0_) * 128
            rks, rows, layer = core_layers[c][sh]
            mj = ((layer == sj) & (rks >= sb0_ * 128)
                  & (rks < sb1_ * 128))
            stream[off + rks[mj] - sb0_ * 128] = rows[mj].astype(np.int16)
            off += n
        idx2s.append(stream)

    # ---- dense-stage feature prep ----
    Xn = features * nsrc[:, None]

    w1p = np.zeros((FP, H), np.float32)
    w1p[:FIN] = W1
    w1p = w1p.astype(BF16)
    w2p = np.zeros((H, 8), np.float32)
    w2p[:, :C] = W2
    w2p = w2p.astype(BF16)
    iota = np.tile(np.arange(W, dtype=np.float32), (128, 1)).astype(BF16)
    identb = np.vstack([np.eye(W, dtype=np.float32)] * 2).astype(BF16)
    b1rep = np.tile(np.asarray(b1, np.float32), (2 * W, 1))     # [128, 128]

    in_maps = []
    for c in range(NCORES):
        gs, dl, key = per_core[c]
        idx_flat = np.zeros(T * 128, np.int64)
        ldst_flat = np.full(T * 128, PAD_DST, np.float32)
        starts = np.zeros(NW * 2 + 1, np.int64)
        starts[1:] = np.cumsum(np.bincount(key, minlength=NW * 2))
        for h in range(2):
            for w in range(NW):
                k = w * 2 + h
                n = starts[k + 1] - starts[k]
                if n == 0:
                    continue
                slot = tile_base[w, h] * 128
                idx_flat[slot:slot + n] = gs[starts[k]:starts[k + 1]] - h * LO
                ldst_flat[slot:slot + n] = dl[starts[k]:starts[k + 1]] % W

        porder = perms[c]
        xnt = np.zeros((FP, NSHP), np.float32)
        xnt[:FIN, :NSH] = Xn[c * NSH + porder].T

        pad_d = np.zeros(NSHP, np.float32)
        pad_d[:NSH] = ndst[c * NSH + porder]
        nsd = pad_d.reshape(NW, W).T
        nsd = np.vstack([nsd, nsd])                             # [128, NW]
        pad_s = np.zeros(NSHP, np.float32)
        pad_s[:NSH] = nsrc[c * NSH + porder]
        nss = pad_s.reshape(NW, W).T
        nss = np.vstack([nss, nss])

        in_maps.append({
            "xnt": xnt.astype(BF16),
            "w1": w1p,
            "w2": w2p,
            "iota": iota,
            "identb": identb,
            "b1rep": b1rep,
            "nsd": np.ascontiguousarray(nsd),
            "nss": np.ascontiguousarray(nss),
            "idx": _wrap_idx(idx_flat),
            "idx2": _wrap_idx(idx2s[c]),
            "ldst": np.ascontiguousarray(
                ldst_flat.reshape(T, 128).T).astype(BF16),
        })
    return (in_maps, tw, tile_win, tile_base, T, gcalls, sched, ndesc,
            perms)


def _build_program(tw, tile_win, T, gcalls, sched, ndesc, timing=False):
    nc = bacc.Bacc("TRN2", target_bir_lowering=False, debug=False,
                   num_devices=NCORES, num_swdge_queues=NSWQ)
    dt = mybir.dt
    xnt_d = nc.dram_tensor("xnt", [FP, NSHP], dt.bfloat16, kind="ExternalInput")
    w1_d = nc.dram_tensor("w1", [FP, H], dt.bfloat16, kind="ExternalInput")
    w2_d = nc.dram_tensor("w2", [H, 8], dt.bfloat16, kind="ExternalInput")
    iota_d = nc.dram_tensor("iota", [128, W], dt.bfloat16, kind="ExternalInput")
    identb_d = nc.dram_tensor("identb", [2 * W, W], dt.bfloat16, kind="ExternalInput")
    b1_d = nc.dram_tensor("b1rep", [2 * W, H], dt.float32, kind="ExternalInput")
    nsd_d = nc.dram_tensor("nsd", [2 * W, NW], dt.float32, kind="ExternalInput")
    nss_d = nc.dram_tensor("nss", [2 * W, NW], dt.float32, kind="ExternalInput")
    idx_d = nc.dram_tensor("idx", [128, T * 8], dt.int16, kind="ExternalInput")
    idx2_d = nc.dram_tensor("idx2", [128, ndesc // 16], dt.int16,
                            kind="ExternalInput")
    ldst_d = nc.dram_tensor("ldst", [128, T], dt.bfloat16, kind="ExternalInput")
    out_d = nc.dram_tensor("out", [128, KB * 8], dt.float32, kind="ExternalOutput")

    with tile.TileContext(nc) as tc:
        with (
            tc.tile_pool(name="const", bufs=1) as cpool,
            tc.tile_pool(name="xnt", bufs=2) as xpool,
            tc.tile_pool(name="g", bufs=GP_BUFS) as gpool,
            tc.tile_pool(name="oh", bufs=OH_BUFS) as ohpool,
            tc.tile_pool(name="ep", bufs=2) as eppool,
            tc.tile_pool(name="small", bufs=2) as spool,
            tc.tile_pool(name="psP", bufs=PSW_BUFS, space="PSUM") as psP,
            tc.tile_pool(name="psT", bufs=1, space="PSUM") as psT,
            tc.tile_pool(name="ps3", bufs=1, space="PSUM") as ps3,
            tc.tile_pool(name="dram", bufs=1, space="DRAM") as dram,
        ):
            # ---- constants ----
            w1_sb = cpool.tile([128, KCH * H], dt.bfloat16, tag="w1")
            nc.sync.dma_start(
                w1_sb[:].rearrange("p (k h) -> p k h", h=H),
                w1_d[:].rearrange("(k p) h -> p k h", p=128))
            w2_sb = cpool.tile([128, 8], dt.bfloat16, tag="w2")
            nc.sync.dma_start(w2_sb[:], w2_d[:])
            iota_sb = cpool.tile([128, W], dt.bfloat16, tag="iota")
            nc.sync.dma_start(iota_sb[:], iota_d[:])
            identb_sb = cpool.tile([2 * W, W], dt.bfloat16, tag="idb")
            nc.sync.dma_start(identb_sb[:], identb_d[:])
            b1_sb = cpool.tile([2 * W, H], dt.float32, tag="b1")
            nc.sync.dma_start(b1_sb[:], b1_d[:])
            nsd_sb = cpool.tile([2 * W, NW], dt.float32, tag="nsd")
            nc.sync.dma_start(nsd_sb[:], nsd_d[:])
            nss_sb = cpool.tile([2 * W, NW], dt.float32, tag="nss")
            nc.sync.dma_start(nss_sb[:], nss_d[:])
            idx_sb = cpool.tile([128, T * 8], dt.int16, tag="idx")
            nc.sync.dma_start(idx_sb[:], idx_d[:])
            idx2_sb = cpool.tile([128, ndesc // 16], dt.int16, tag="idx2")
            nc.sync.dma_start(idx2_sb[:], idx2_d[:])
            ldst_sb = cpool.tile([128, T], dt.bfloat16, tag="ldst")
            nc.sync.dma_start(ldst_sb[:], ldst_d[:])
            x2stage = cpool.tile([128, KB * 8], dt.float32, tag="x2st")
            zr = cpool.tile([128, KB * 8], dt.float32, tag="zr")
            nc.vector.memset(zr[:], 0.0)

            ag_h_in = dram.tile([NSHP, H], dt.bfloat16)
            h_full = dram.tile([TROWS, H], dt.bfloat16, addr_space="Shared")
            tbl = dram.tile([TROWS, STEP], dt.float32)
            rs_out = dram.tile([NSHP, STEP], dt.float32)

            # zero the scatter table payload columns (cols 0:8 of each row)
            for k in range(NCORES):
                nc.sync.dma_start(
                    tbl[k * NSHP:(k + 1) * NSHP, 0:8].rearrange(
                        "(b p) f -> p b f", p=128),
                    zr[:].rearrange("p (b f) -> p b f", f=8))

            # ---- stage B: h_sh = XnT_sh.T @ W1 ----
            for sb0 in range(0, KB, NSB):
                nsb = min(NSB, KB - sb0)
                xnt_sb = xpool.tile([128, KCH * NSB * 128], dt.bfloat16,
                                    tag="xnt")
                nc.sync.dma_start(
                    xnt_sb[:, :KCH * nsb * 128].rearrange(
                        "p (k n) -> p k n", k=KCH),
                    xnt_d[:, sb0 * 128:(sb0 + nsb) * 128].rearrange(
                        "(k p) n -> p k n", p=128))
                for nb in range(nsb):
                    ph = psP.tile([2 * W, H], dt.float32, tag="pw")
                    for k in range(KCH):
                        nc.tensor.matmul(
                            out=ph[:],
                            lhsT=xnt_sb[:, (k * nsb + nb) * 128:
                                        (k * nsb + nb) * 128 + 128],
                            rhs=w1_sb[:, k * H:(k + 1) * H],
                            start=(k == 0), stop=(k == KCH - 1))
                    hb = spool.tile([128, H], dt.bfloat16, tag="hb")
                    nc.vector.tensor_copy(out=hb[:], in_=ph[:])
                    nc.sync.dma_start(
                        ag_h_in[(sb0 + nb) * 128:(sb0 + nb) * 128 + 128, :],
                        hb[:])

            if timing:
                nc.sync.dma_start(h_full[0:NSHP, :], ag_h_in[:])
            else:
                nc.gpsimd.collective_compute(
                    "AllGather", mybir.AluOpType.bypass,
                    replica_groups=[list(range(NCORES))],
                    ins=[ag_h_in[:].opt()], outs=[h_full[:].opt()])

            # ---- layer-1 aggregation, window-major ----
            def win_epilogue(w, psum):
                po = (w % 2) * W
                co8 = (w // 2) * 8
                ps = psum[po:po + W, :]
                u = eppool.tile([2 * W, H], dt.float32, tag="ep")
                uh = u[po:po + W, :]
                nc.vector.tensor_scalar(
                    out=uh, in0=ps, scalar1=nsd_sb[po:po + W, w:w + 1],
                    scalar2=None, op0=mybir.AluOpType.mult)
                nc.vector.tensor_tensor(
                    out=uh, in0=uh, in1=b1_sb[po:po + W, :],
                    op=mybir.AluOpType.add)
                vb = eppool.tile([2 * W, H], dt.bfloat16, tag="vb")
                nc.vector.tensor_scalar(
                    out=vb[po:po + W, :], in0=uh,
                    scalar1=nss_sb[po:po + W, w:w + 1],
                    scalar2=0.0, op0=mybir.AluOpType.mult,
                    op1=mybir.AluOpType.max)
                pt = psT.tile([128, W], dt.bfloat16, tag="pt")
                nc.tensor.transpose(out=pt[:], in_=vb[po:po + W, :],
                                    identity=identb_sb[po:po + W, :])
                at = spool.tile([128, W], dt.bfloat16, tag="at")
                nc.vector.tensor_copy(out=at[:], in_=pt[:])
                p3 = ps3.tile([2 * W, 8], dt.float32, tag="p3")
                nc.tensor.matmul(out=p3[po:po + W, :], lhsT=at[:],
                                 rhs=w2_sb[:], start=True, stop=True)
                nc.vector.tensor_copy(
                    out=x2stage[po:po + W, co8:co8 + 8],
                    in_=p3[po:po + W, :])

            # ---- layer-2 scatter call emission (interleaved with L1) ----
            sched_pos = [0]
            off16s = np.concatenate(
                [[0], np.cumsum([(e[3] - e[2]) * 8 for e in sched])])

            def emit_ready_scatters(closed):
                while sched_pos[0] < len(sched):
                    h2, j2, b0_, b1_ = sched[sched_pos[0]]
                    if b1_ * 2 > closed:
                        break
                    n = (b1_ - b0_) * 128
                    o16 = int(off16s[sched_pos[0]])
                    out_view = (tbl[0:SPLIT, 0:8] if h2 == 0
                                else tbl[SPLIT:TROWS, 0:8])
                    nc.gpsimd.dma_scatter_add(
                        out_view,
                        x2stage[:, b0_ * 8:b1_ * 8].rearrange(
                            "p (n e) -> p n e", e=8),
                        idx2_sb[:, o16:o16 + n // 16],
                        n, n, 8,
                        elem_step=STEP,
                        queue_num=0)
                    sched_pos[0] += 1

            closedw = np.zeros(NW, bool)
            closed_upto = [0]

            def mark_closed(w):
                closedw[w] = True
                while closed_upto[0] < NW and closedw[closed_upto[0]]:
                    closed_upto[0] += 1
                emit_ready_scatters(closed_upto[0])

            wtot = tw[:, 0] + tw[:, 1]
            nproc = np.zeros(NW, np.int64)
            pw = {}      # window -> (psum tile, nmm)
            for hcall, t0, ntc in gcalls:
                tbl_h = h_full[0:LO, :] if hcall == 0 else h_full[LO:TROWS, :]
                gt = gpool.tile([128, BT * H], dt.bfloat16, tag="g")
                nc.gpsimd.dma_gather(
                    out_ap=gt[:, :ntc * H].rearrange("p (n e) -> p n e", e=H),
                    in_ap=tbl_h,
                    idxs_ap=idx_sb[:, t0 * 8:(t0 + ntc) * 8],
                    num_idxs=ntc * 128,
                    num_idxs_reg=ntc * 128,
                    elem_size=H,
                    single_packet=False,
                    queue_num=0)
                oh = ohpool.tile([128, BT * W], dt.bfloat16, tag="oh")
                ld = ldst_sb[:, t0:t0 + ntc]
                nc.vector.tensor_tensor(
                    out=oh[:, :ntc * W].rearrange("p (n w) -> p n w", w=W),
                    in0=bass.AP(ld.tensor, ld.offset, ld.ap + [[0, W]]),
                    in1=bass.AP(iota_sb[:].tensor, iota_sb[:].offset,
                                [iota_sb[:].ap[0], [0, ntc],
                                 iota_sb[:].ap[1]]),
                    op=mybir.AluOpType.is_equal)
                for j in range(ntc):
                    t = t0 + j
                    w = tile_win[t]
                    po = (w % 2) * W
                    if w not in pw:
                        pw[w] = [psP.tile([2 * W, H], dt.float32,
                                          name=f"pw{w}", tag="pw"), 0]
                    last = nproc[w] + 1 == wtot[w]
                    nc.tensor.matmul(
                        out=pw[w][0][po:po + W, :],
                        lhsT=oh[:, j * W:(j + 1) * W],
                        rhs=gt[:, j * H:(j + 1) * H],
                        start=(pw[w][1] == 0), stop=bool(last))
                    pw[w][1] += 1
                    nproc[w] += 1
                    if last:
                        win_epilogue(w, pw.pop(w)[0])
                        mark_closed(w)
            assert not pw, f"unclosed windows {list(pw)}"
            emit_ready_scatters(NW)
            assert sched_pos[0] == len(sched)

            if timing:
                nc.sync.dma_start(rs_out[:], tbl[0:NSHP, :])
            else:
                nc.gpsimd.collective_compute(
                    "ReduceScatter", mybir.AluOpType.add,
                    replica_groups=[list(range(NCORES))],
                    ins=[tbl[:].opt()], outs=[rs_out[:].opt()])

            # ---- finish: out = rs * ndst  (wrap layout [128, 49*8]) ----
            # nfin8[p, b*8+f] = ndst at rank b*128+p; equals nsd rearranged,
            # but cheaper to just recompute from nsd on host -- kept as
            # device-side multiply against a dedicated const
            nfin_d = nc.dram_tensor("nfin8", [128, KB * 8], dt.float32,
                                    kind="ExternalInput")
            nfin_sb = cpool.tile([128, KB * 8], dt.float32, tag="nfin")
            nc.sync.dma_start(nfin_sb[:], nfin_d[:])
            fin = cpool.tile([128, KB * 8], dt.float32, tag="fin")
            nc.sync.dma_start(
                fin[:].rearrange("p (b f) -> p b f", f=8),
                rs_out[:, 0:8].rearrange("(b p) f -> p b f", p=128))
            nc.vector.tensor_tensor(
                out=fin[:], in0=fin[:], in1=nfin_sb[:],
                op=mybir.AluOpType.mult)
            nc.sync.dma_start(out_d[:], fin[:])
    nc.compile()
    return nc


_CACHE = {}
_LAST_RES = None


def kernel(features, src, dst, W1, b1, W2, b2):
    (in_maps, tw, tile_win, tile_base, T, gcalls, sched, ndesc,
     perms) = _prep(features, src, dst, W1, b1, W2, b2)
    # nfin8 const: ndst (permuted, padded) in wrap layout
    for c in range(NCORES):
        nsd = in_maps[c]["nsd"]     # [128, NW]: nsd[po+r, w] = nd[w*64+r]
        pad_d = np.empty(NSHP, np.float32)
        for w in range(NW):
            pad_d[w * W:(w + 1) * W] = nsd[(w % 2) * W:(w % 2) * W + W, w]
        v = pad_d.reshape(KB, 128).T
        in_maps[c]["nfin8"] = np.ascontiguousarray(
            np.repeat(v[:, :, None], 8, axis=2).reshape(128, KB * 8)
            .astype(np.float32))
    key = (T, tuple(tw.reshape(-1).tolist()), tuple(gcalls), tuple(sched))
    if key not in _CACHE:
        _CACHE[key] = _build_program(tw, tile_win, T, gcalls, sched, ndesc)
    nc = _CACHE[key]
    res = bass_utils.run_bass_kernel_spmd(
        nc, in_maps, core_ids=list(range(NCORES)))
    global _LAST_RES
    _LAST_RES = res
    out = np.empty((N, C), np.float32)
    b2f = np.asarray(b2, np.float32)
    for c in range(NCORES):
        arr = res.results[c]["out"].reshape(128, KB, 8)
        by_rank = arr.transpose(1, 0, 2).reshape(NSHP, 8)
        out[c * NSH + perms[c]] = by_rank[:NSH, :C]
    out += b2f[None, :]
    return out
